# revision 10
# baseline (speedup 1.0000x reference)
"""Trainium2 SPMD kernel for nn_AutoCorrelation_loss_V (sparse_attention).

Math summary (reference reduces to this exactly):
  - scores are constant along the unmasked (causal) key range, so softmax is
    uniform over l <= index[k]: attn @ V == prefix-mean of V at the selected rows.
  - the output is cumsum(V, axis=L) with the 7 selected rows divided by (idx+1).
  - the top-7 indices come from corr.mean(batch), where
      corr[b,t] = 0.25*(LSE_i1 + LSE_i2 + LSE_t1 + LSE_t2) - <q[b,t], k[b,t]>
    with LSE_t* = row-logsumexp (diag dropped) of the temporal Gram
    Z_b @ Z_b^T (Z_b = concat(q_b, k_b), [4096, 512]) and LSE_i* the row-LSE of
    the per-timestep 8x8 instance Gram.

Sharding (8 cores): core c = (b = c//2, half = c%2)
  - temporal Gram rows [2048*half : 2048*half+2048) of batch b: PE matmuls in
    float32r, fused exp(x - 100) + chunk-sum on ScalarE -> "esums" output.
  - instance grams for t in [256c, 256c+256): DVE rowwise-dot -> "eslab" output.
  - cumsum of V planes (b, heads 4*half..4*half+4) via triangular-ones matmuls
    (hierarchical: chunk-local cumsum + chunk-sum carry) -> "planes" output.
Host: combines the tiny LSE partials, takes top-7, divides those 7 rows by
(idx+1) while assembling the full [4, 8, 2048, 64] output.
"""

import sys

import numpy as np

sys.path.insert(0, "/opt/trn_rl_repo")

import concourse.bacc as bacc
import concourse.tile as tile
from concourse import mybir
from concourse.bass_utils import run_bass_kernel_spmd

F32 = mybir.dt.float32
F32R = mybir.dt.float32r

B, L, H, E = 4, 2048, 8, 64
C = H * E  # 512
T2 = 2 * L  # 4096
NCORES = 8
TOPK = 7  # int(1.0 * log(2048))
SHIFT = 100.0  # global exp shift; temporal Gram entries are in [-180, 180]

LAST_RUN = None  # BassKernelResults of the most recent launch (for test.py)

_CACHED = {}


def _build_nc():
    nc = bacc.Bacc("TRN2", target_bir_lowering=False, debug=False,
                   num_devices=NCORES)

    zt_d = nc.dram_tensor("zt", [4, 128, T2], F32R, kind="ExternalInput").ap()
    zi_d = nc.dram_tensor("zi", [2, 128, 8, C], F32, kind="ExternalInput").ap()
    vp_d = nc.dram_tensor("vp", [128, 16, 4, E], F32R, kind="ExternalInput").ap()
    triu_d = nc.dram_tensor("triu", [128, 128], F32R, kind="ExternalInput").ap()
    ohw_d = nc.dram_tensor("ohw", [128, 31], F32R, kind="ExternalInput").ap()
    ltw_d = nc.dram_tensor("ltw", [16, 2048], F32R, kind="ExternalInput").ap()
    imask_d = nc.dram_tensor("imask", [128, 128], F32, kind="ExternalInput").ap()

    esums_d = nc.dram_tensor("esums", [128, 16, 4], F32, kind="ExternalOutput").ap()
    eslab_d = nc.dram_tensor("eslab", [2, 128, 36], F32, kind="ExternalOutput").ap()
    planes_d = nc.dram_tensor("planes", [128, 16, 256], F32, kind="ExternalOutput").ap()

    with tile.TileContext(nc) as tc:
        with tc.tile_pool(name="zt", bufs=1) as ztp, \
             tc.tile_pool(name="zi", bufs=1) as zip_, \
             tc.tile_pool(name="vp", bufs=1) as vpp, \
             tc.tile_pool(name="const", bufs=1) as cp, \
             tc.tile_pool(name="small", bufs=1) as smp, \
             tc.tile_pool(name="scr", bufs=3) as scp, \
             tc.tile_pool(name="iscr", bufs=2) as iscp, \
             tc.tile_pool(name="osb", bufs=3) as osp, \
             tc.tile_pool(name="gram", bufs=2, space="PSUM") as gp, \
             tc.tile_pool(name="cum", bufs=2, space="PSUM") as cump, \
             tc.tile_pool(name="sums", bufs=1, space="PSUM") as sump:

            triu_sb = cp.tile([128, 128], F32R, tag="triu")
            nc.sync.dma_start(triu_sb[:], triu_d)
            ohw_sb = cp.tile([128, 31], F32R, tag="ohw")
            nc.sync.dma_start(ohw_sb[:], ohw_d)
            ltw_sb = cp.tile([16, 2048], F32R, tag="ltw")
            nc.sync.dma_start(ltw_sb[:], ltw_d)
            imask_sb = cp.tile([128, 128], F32, tag="imask")
            nc.sync.dma_start(imask_sb[:], imask_d)
            bias_sb = cp.tile([128, 1], F32, tag="bias")
            nc.gpsimd.memset(bias_sb[:], -SHIFT)

            # DMA order tuned for overlap: zt chunk-columns n=0,1 first so the
            # temporal matmuls can start ~3us in; vp next (cumsum block runs
            # between temporal passes); the rest of zt streams behind.
            zt_sb = []
            for kk in range(4):
                t = ztp.tile([128, T2], F32R, tag=f"zt{kk}")
                zt_sb.append(t)

            def load_zt_chunk(n):
                for kk in range(4):
                    nc.sync.dma_start(zt_sb[kk][:, 512 * n:512 * n + 512],
                                      zt_d[kk, :, 512 * n:512 * n + 512])

            load_zt_chunk(0)
            load_zt_chunk(1)
            vp_sb = vpp.tile([128, 16, 4, E], F32R, tag="vp")
            nc.sync.dma_start(vp_sb[:], vp_d)
            load_zt_chunk(2)
            load_zt_chunk(3)
            zi_sb = []
            for tt in range(2):
                t = zip_.tile([128, 8, C], F32, tag=f"zi{tt}")
                nc.sync.dma_start(t[:], zi_d[tt])
                zi_sb.append(t)
            for n in range(4, 8):
                load_zt_chunk(n)

            esums_sb = smp.tile([128, 16, 4], F32, tag="esums")

            # ---- temporal Gram + fused exp/chunk-sum, n-pair np at a time ----
            def temporal_pass(np_):
                for m in range(16):
                    ps = gp.tile([128, 1024], F32, tag="gram")
                    for hn in range(2):
                        n = 2 * np_ + hn
                        for kk in range(4):
                            nc.tensor.matmul(
                                ps[:, 512 * hn:512 * hn + 512],
                                zt_sb[kk][:, 128 * m:128 * m + 128],
                                zt_sb[kk][:, 512 * n:512 * n + 512],
                                start=(kk == 0), stop=(kk == 3))
                    if (m // 4) // 2 == np_:
                        off = 512 * ((m // 4) % 2) + 128 * (m % 4)
                        nc.vector.tensor_mul(ps[:, off:off + 128],
                                             ps[:, off:off + 128], imask_sb[:])
                    scr = scp.tile([128, 1024], F32, tag="scr")
                    nc.scalar.activation(scr[:], ps[:],
                                         mybir.ActivationFunctionType.Exp,
                                         bias=bias_sb[:],
                                         accum_out=esums_sb[:, m, np_:np_ + 1])

            temporal_pass(0)

            # ---- cumsum of V planes (PE-cheap; sits between temporal passes)
            ps_sums = sump.tile([16, 256], F32, tag="sums")
            for n in range(16):
                nc.tensor.matmul(ps_sums[:], ohw_sb[:, 15 - n:31 - n],
                                 vp_sb[:, n], start=(n == 0), stop=(n == 15))
            sums_sb = smp.tile([16, 256], F32R, tag="sums_sb")
            nc.scalar.copy(sums_sb[:], ps_sums[:])

            for n in range(16):
                pc = cump.tile([128, 256], F32, tag="pc")
                nc.tensor.matmul(pc[:], ltw_sb[:, 128 * n:128 * n + 128],
                                 sums_sb[:], start=True, stop=False)
                nc.tensor.matmul(pc[:], triu_sb[:], vp_sb[:, n],
                                 start=False, stop=True)
                out_sb = osp.tile([128, 256], F32, tag="osb")
                nc.scalar.copy(out_sb[:], pc[:])
                nc.sync.dma_start(planes_d[:, n], out_sb[:])

            for np_ in range(1, 4):
                temporal_pass(np_)
            nc.sync.dma_start(esums_d, esums_sb[:])

            # ---- instance grams: rowwise dots on DVE (36 pairs, host mirrors)
            pairs = [(i, j) for i in range(8) for j in range(i, 8)]
            for tt in range(2):
                eslab_sb = smp.tile([128, 36], F32, tag=f"eslab{tt}")
                for p, (i, j) in enumerate(pairs):
                    iscr = iscp.tile([128, C], F32, tag="iscr")
                    nc.vector.scalar_tensor_tensor(
                        iscr[:], zi_sb[tt][:, i, :], 1.0,
                        zi_sb[tt][:, j, :],
                        op0=mybir.AluOpType.mult,
                        op1=mybir.AluOpType.mult,
                        accum_out=eslab_sb[:, p:p + 1])
                nc.sync.dma_start(eslab_d[tt], eslab_sb[:])

    nc.compile()
    return nc


def _consts():
    k = np.arange(128)
    triu = (k[:, None] <= k[None, :]).astype(np.float32)          # lhsT cumsum
    ohw = np.zeros((128, 31), np.float32)
    ohw[:, 15] = 1.0                                              # one-hot cols
    cc = np.arange(16)
    nn = np.arange(2048) // 128
    ltw = (cc[:, None] < nn[None, :]).astype(np.float32)          # carry mask
    imask = (1.0 - np.eye(128)).astype(np.float32)
    return triu, ohw, ltw, imask


def prepare_in_maps(queries, keys, values):
    q = np.ascontiguousarray(queries, dtype=np.float32).reshape(B, L, C)
    k = np.ascontiguousarray(keys, dtype=np.float32).reshape(B, L, C)
    v = np.ascontiguousarray(values, dtype=np.float32)            # [B,L,H,E]

    triu, ohw, ltw, imask = _consts()
    # Zi[i] = q[i] for i<B else k[i-B]  -> [2B, L, C]
    Zi = np.concatenate([q, k], axis=0)

    in_maps = []
    for c in range(NCORES):
        b, half = c // 2, c % 2
        Zb = np.concatenate([q[b], k[b]], axis=0)                 # [4096, 512]
        own = Zb[2048 * half:2048 * half + 2048]
        oth = Zb[2048 * (1 - half):2048 * (1 - half) + 2048]
        zt = np.ascontiguousarray(
            np.concatenate([own, oth], axis=0).T).reshape(4, 128, T2)
        t0 = 256 * c
        zi = np.ascontiguousarray(
            Zi[:, t0:t0 + 256, :].transpose(1, 0, 2)).reshape(2, 128, 8, C)
        vp = np.ascontiguousarray(
            v[b].reshape(16, 128, H, E)[:, :, 4 * half:4 * half + 4, :]
            .transpose(1, 0, 2, 3))                               # [128,16,4,64]
        in_maps.append({
            "zt": zt, "zi": zi, "vp": vp,
            "triu": triu, "ohw": ohw, "ltw": ltw, "imask": imask,
        })
    return in_maps


def get_nc():
    if "nc" not in _CACHED:
        _CACHED["nc"] = _build_nc()
    return _CACHED["nc"]


def kernel(queries, keys, values, attn_mask):
    global LAST_RUN
    nc = get_nc()
    in_maps = prepare_in_maps(queries, keys, values)

    res = run_bass_kernel_spmd(nc, in_maps, list(range(NCORES)))
    LAST_RUN = res
    results = res.results

    # ---- host combine (tiny) ----
    # temporal LSE: lse[r] = log(sum_n esums[p, m, n]) + SHIFT, r = 128m + p
    lse_t = np.zeros((B, 2, L))                                   # [b, half, t]
    dots = np.zeros((B, L))
    li_sum = np.zeros(L)                                          # sum_i LSE_inst
    pairs = [(i, j) for i in range(8) for j in range(i, 8)]
    for c in range(NCORES):
        b, half = c // 2, c % 2
        r = results[c]
        s = r["esums"].astype(np.float64).sum(axis=2)             # [128, 16]
        lse_t[b, half] = np.log(s).T.reshape(L) + SHIFT
        epk = r["eslab"].astype(np.float64).reshape(256, 36)      # [t, pair]
        e = np.empty((256, 8, 8))
        for p, (i, j) in enumerate(pairs):
            e[:, i, j] = epk[:, p]
            e[:, j, i] = epk[:, p]
        t0 = 256 * c
        for bb in range(B):
            dots[bb, t0:t0 + 256] = e[:, bb, 4 + bb]
        ii = np.arange(8)
        e[:, ii, ii] = -np.inf
        m = e.max(axis=2, keepdims=True)
        li = np.log(np.exp(e - m).sum(axis=2)) + m[..., 0]        # [256, 8]
        li_sum[t0:t0 + 256] = li.sum(axis=1)

    corr_mean = (li_sum + lse_t.sum(axis=(0, 1))) / 16.0 - dots.mean(axis=0)
    index = np.argsort(-corr_mean, kind="stable")[:TOPK]

    out = np.empty((B, H, L, E), np.float32)
    for c in range(NCORES):
        b, half = c // 2, c % 2
        pl = results[c]["planes"].reshape(128, 16, 4, E)
        out[b, 4 * half:4 * half + 4] = (
            pl.transpose(2, 1, 0, 3).reshape(4, L, E))
    out[:, :, index, :] /= (index + 1).astype(np.float32)[None, None, :, None]
    return out


# revision 36
# speedup vs baseline: 69.5736x; 69.5736x over previous
"""Trainium2 SPMD kernel for nn_AutoCorrelation_loss_V (sparse_attention).

Math summary (reference reduces to this exactly):
  - scores are constant along the unmasked (causal) key range, so softmax is
    uniform over l <= index[k]: attn @ V == prefix-mean of V at the selected rows.
  - the output is cumsum(V, axis=L) with the 7 selected rows divided by (idx+1).
  - the top-7 indices come from corr.mean(batch), where
      corr[b,t] = 0.25*(LSE_i1 + LSE_i2 + LSE_t1 + LSE_t2) - <q[b,t], k[b,t]>
    with LSE_t* = row-logsumexp (diag dropped) of the temporal Gram
    Z_b @ Z_b^T (Z_b = concat(q_b, k_b), [4096, 512]) and LSE_i* the row-LSE of
    the per-timestep 8x8 instance Gram.

Sharding (8 cores): core c = (b = c//2, half = c%2)
  - temporal Gram rows [2048*half : 2048*half+2048) of batch b: PE matmuls in
    float32r, fused exp(x - 100) + chunk-sum on ScalarE -> "esums" output.
  - instance grams for t in [256c, 256c+256): DVE rowwise-dot -> "eslab" output.
  - cumsum of V planes (b, heads 4*half..4*half+4) via triangular-ones matmuls
    (hierarchical: chunk-local cumsum + chunk-sum carry) -> "planes" output.
Host: combines the tiny LSE partials, takes top-7, divides those 7 rows by
(idx+1) while assembling the full [4, 8, 2048, 64] output.
"""

import sys

import numpy as np

sys.path.insert(0, "/opt/trn_rl_repo")

import concourse.bacc as bacc
import concourse.tile as tile
from concourse import mybir
from concourse.bass_utils import run_bass_kernel_spmd

F32 = mybir.dt.float32
F32R = mybir.dt.float32r
BF16 = mybir.dt.bfloat16

B, L, H, E = 4, 2048, 8, 64
C = H * E  # 512
T2 = 2 * L  # 4096
NCORES = 8
TOPK = 7  # int(1.0 * log(2048))
SHIFT = 100.0  # global exp shift; temporal Gram entries are in [-180, 180]

LAST_RUN = None  # BassKernelResults of the most recent launch (for test.py)

_CACHED = {}


def _build_nc():
    nc = bacc.Bacc("TRN2", target_bir_lowering=False, debug=False,
                   num_devices=NCORES)

    zt_d = nc.dram_tensor("zt", [4, 128, T2], F32R, kind="ExternalInput").ap()
    zi_d = nc.dram_tensor("zi", [2, 128, 8, C], F32, kind="ExternalInput").ap()
    vp_d = nc.dram_tensor("vp", [128, 16, 4, E], F32R, kind="ExternalInput").ap()
    triu_d = nc.dram_tensor("triu", [128, 128], F32R, kind="ExternalInput").ap()
    ohw_d = nc.dram_tensor("ohw", [128, 31], F32R, kind="ExternalInput").ap()
    ohwb_d = nc.dram_tensor("ohwb", [128, 31], BF16, kind="ExternalInput").ap()
    ltw_d = nc.dram_tensor("ltw", [16, 2048], F32R, kind="ExternalInput").ap()
    imask_d = nc.dram_tensor("imask", [128, 128], F32, kind="ExternalInput").ap()

    esums_d = nc.dram_tensor("esums", [128, 16, 3], F32, kind="ExternalOutput").ap()
    csums_d = nc.dram_tensor("csums", [14, 512], F32, kind="ExternalOutput").ap()
    eslab_d = nc.dram_tensor("eslab", [2, 128, 36], F32, kind="ExternalOutput").ap()
    planes_d = nc.dram_tensor("planes", [128, 16, 256], F32, kind="ExternalOutput").ap()

    with tile.TileContext(nc) as tc:
        with tc.tile_pool(name="zt", bufs=1) as ztp, \
             tc.tile_pool(name="zi", bufs=1) as zip_, \
             tc.tile_pool(name="vp", bufs=1) as vpp, \
             tc.tile_pool(name="const", bufs=1) as cp, \
             tc.tile_pool(name="small", bufs=1) as smp, \
             tc.tile_pool(name="scr", bufs=4) as scp, \
             tc.tile_pool(name="iscr", bufs=2) as iscp, \
             tc.tile_pool(name="osb", bufs=3) as osp, \
             tc.tile_pool(name="gram", bufs=2, space="PSUM") as gp, \
             tc.tile_pool(name="ghalf", bufs=3, space="PSUM") as gph, \
             tc.tile_pool(name="csp", bufs=1, space="PSUM") as csp:

            triu_sb = cp.tile([128, 128], F32R, tag="triu")
            nc.sync.dma_start(triu_sb[:], triu_d)
            ohw_sb = cp.tile([128, 31], F32R, tag="ohw")
            nc.sync.dma_start(ohw_sb[:], ohw_d)
            ohwb_sb = cp.tile([128, 31], BF16, tag="ohwb")
            nc.sync.dma_start(ohwb_sb[:], ohwb_d)
            ltw_sb = cp.tile([16, 2048], F32R, tag="ltw")
            nc.sync.dma_start(ltw_sb[:], ltw_d)
            imask_sb = cp.tile([128, 128], F32, tag="imask")
            nc.sync.dma_start(imask_sb[:], imask_d)
            bias_sb = cp.tile([128, 1], F32, tag="bias")
            nc.gpsimd.memset(bias_sb[:], -SHIFT)

            # DMA order tuned for overlap: zt chunk-columns n=0,1 first so the
            # temporal matmuls can start ~3us in; vp next (cumsum block runs
            # between temporal passes); the rest of zt streams behind.
            zt_sb = []
            for kk in range(4):
                t = ztp.tile([128, T2], F32R, tag=f"zt{kk}")
                zt_sb.append(t)

            def load_zt_chunk(n):
                for kk in range(4):
                    nc.sync.dma_start(zt_sb[kk][:, 512 * n:512 * n + 512],
                                      zt_d[kk, :, 512 * n:512 * n + 512])

            load_zt_chunk(0)
            load_zt_chunk(1)
            vp_sb = vpp.tile([128, 16, 4, E], F32R, tag="vp")
            nc.sync.dma_start(vp_sb[:], vp_d)
            load_zt_chunk(2)
            load_zt_chunk(3)
            zi_sb = []
            for tt in range(2):
                t = zip_.tile([128, 8, C], F32, tag=f"zi{tt}")
                nc.sync.dma_start(t[:], zi_d[tt])
                zi_sb.append(t)
            for n in range(4, 8):
                load_zt_chunk(n)

            esums_sb = smp.tile([128, 16, 3], F32, tag="esums")
            # colsum accumulator rows: 0..5 = own-half skipped lower
            # super-blocks (pair p = (r, c), r < c, row-major); 6..13 = cross
            # checkerboard sub-blocks (p = 6 + 2*r' + hb)
            pairs_rc = [(0, 1), (0, 2), (0, 3), (1, 2), (1, 3), (2, 3)]
            cs_ps = csp.tile([14, 512], F32, tag="csps")
            cs_state = {"first": True, "left": 24 + 32, "pending": []}

            def colsum_mm(p, rhs_ap):
                # deferred one unit so the PE (in-order) never waits on the
                # ACT exp that produces rhs
                cs_state["pending"].append((p, rhs_ap))

            def flush_colsums(keep=1):
                while len(cs_state["pending"]) > keep:
                    p, rhs_ap = cs_state["pending"].pop(0)
                    nc.tensor.matmul(cs_ps[:], ohwb_sb[:, 15 - p:29 - p],
                                     rhs_ap,
                                     start=cs_state["first"],
                                     stop=cs_state["left"] == 1,
                                     skip_group_check=True)
                    cs_state["first"] = False
                    cs_state["left"] -= 1

            # ---- temporal Gram + fused exp/chunk-sum, n-pair np at a time ----
            # Own-half columns (chunks 0..3) use upper-triangular symmetry:
            # chunk n < m//4 is skipped; its row-sums are recovered from
            # column-sums of the mirrored exp'd block (csums).
            def temporal_pass(np_):
                for m in range(16):
                    g = m // 4
                    halves = [hn for hn in (0, 1) if 2 * np_ + hn >= g]
                    if not halves:
                        continue
                    flush_colsums(keep=4)
                    full = len(halves) == 2
                    width = 1024 if full else 512
                    ps = (gp.tile([128, width], F32, tag="gram", name="ps")
                          if full else
                          gph.tile([128, width], F32, tag="ghalf", name="ps"))
                    for hn in halves:
                        n = 2 * np_ + hn
                        col0 = 512 * (hn - halves[0])
                        for kk in range(4):
                            nc.tensor.matmul(
                                ps[:, col0:col0 + 512],
                                zt_sb[kk][:, 128 * m:128 * m + 128],
                                zt_sb[kk][:, 512 * n:512 * n + 512],
                                start=(kk == 0), stop=(kk == 3))
                    if 2 * np_ <= g <= 2 * np_ + 1:
                        # diagonal block lives in chunk n == g (first computed
                        # half): zero it so exp gives 0 (e^-100 underflows)
                        off = 128 * (m % 4)
                        nc.vector.tensor_mul(ps[:, off:off + 128],
                                             ps[:, off:off + 128], imask_sb[:])
                    scr = scp.tile([128, width], BF16,
                                   tag="scr" if full else "scrh")
                    nc.scalar.activation(scr[:], ps[:],
                                         mybir.ActivationFunctionType.Exp,
                                         bias=bias_sb[:],
                                         accum_out=esums_sb[:, m, np_:np_ + 1])
                    # column sums for strictly-upper own-half blocks (g < n < 4)
                    for hn in halves:
                        n = 2 * np_ + hn
                        if n < 4 and n > g:
                            p = pairs_rc.index((g, n))
                            col0 = 512 * (hn - halves[0])
                            colsum_mm(p, scr[:, col0:col0 + 512])

            # cross block (own rows x other-half cols): checkerboard — this
            # core computes col-positions cpos with (r' + cpos) even (the
            # other core's input rotation makes it cover the odd ones), and
            # emits colsums so the mirror rows are recovered host-side.
            def cross_pass():
                for m in range(16):
                    rp = m // 4
                    flush_colsums(keep=4)
                    ps = gp.tile([128, 1024], F32, tag="gram", name="ps")
                    for hb in range(2):
                        n = 4 + (rp % 2) + 2 * hb
                        for kk in range(4):
                            nc.tensor.matmul(
                                ps[:, 512 * hb:512 * hb + 512],
                                zt_sb[kk][:, 128 * m:128 * m + 128],
                                zt_sb[kk][:, 512 * n:512 * n + 512],
                                start=(kk == 0), stop=(kk == 3))
                    scr = scp.tile([128, 1024], BF16, tag="scr", name="scr")
                    nc.scalar.activation(scr[:], ps[:],
                                         mybir.ActivationFunctionType.Exp,
                                         bias=bias_sb[:],
                                         accum_out=esums_sb[:, m, 2:3])
                    for hb in range(2):
                        colsum_mm(6 + 2 * rp + hb,
                                  scr[:, 512 * hb:512 * hb + 512])

            temporal_pass(0)

            # ---- cumsum of V planes (PE-cheap; sits between temporal passes)
            ps_sums = gph.tile([16, 256], F32, tag="ghalf")
            for n in range(16):
                nc.tensor.matmul(ps_sums[:], ohw_sb[:, 15 - n:31 - n],
                                 vp_sb[:, n], start=(n == 0), stop=(n == 15))
            sums_sb = smp.tile([16, 256], F32R, tag="sums_sb")
            nc.scalar.copy(sums_sb[:], ps_sums[:])

            for n in range(16):
                pc = gph.tile([128, 256], F32, tag="ghalf")
                nc.tensor.matmul(pc[:], ltw_sb[:, 128 * n:128 * n + 128],
                                 sums_sb[:], start=True, stop=False)
                nc.tensor.matmul(pc[:], triu_sb[:], vp_sb[:, n],
                                 start=False, stop=True)
                out_sb = osp.tile([128, 256], F32, tag="osb")
                nc.scalar.copy(out_sb[:], pc[:])
                nc.sync.dma_start(planes_d[:, n], out_sb[:])

            temporal_pass(1)
            cross_pass()
            flush_colsums(keep=0)
            csums_sb = smp.tile([14, 512], F32, tag="csums_sb")
            nc.scalar.copy(csums_sb[:], cs_ps[:])
            nc.sync.dma_start(csums_d, csums_sb[:])
            nc.sync.dma_start(esums_d, esums_sb[:])

            # ---- instance grams: rowwise dots on DVE (36 pairs, host mirrors)
            pairs = [(i, j) for i in range(8) for j in range(i, 8)]
            for tt in range(2):
                eslab_sb = smp.tile([128, 36], F32, tag=f"eslab{tt}")
                for p, (i, j) in enumerate(pairs):
                    iscr = iscp.tile([128, C], F32, tag="iscr")
                    nc.vector.scalar_tensor_tensor(
                        iscr[:], zi_sb[tt][:, i, :], 1.0,
                        zi_sb[tt][:, j, :],
                        op0=mybir.AluOpType.mult,
                        op1=mybir.AluOpType.mult,
                        accum_out=eslab_sb[:, p:p + 1])
                nc.sync.dma_start(eslab_d[tt], eslab_sb[:])

    nc.compile()
    return nc


def _consts():
    k = np.arange(128)
    triu = (k[:, None] <= k[None, :]).astype(np.float32)          # lhsT cumsum
    ohw = np.zeros((128, 31), np.float32)
    ohw[:, 15] = 1.0                                              # one-hot cols
    cc = np.arange(16)
    nn = np.arange(2048) // 128
    ltw = (cc[:, None] < nn[None, :]).astype(np.float32)          # carry mask
    imask = (1.0 - np.eye(128)).astype(np.float32)
    import ml_dtypes
    ohwb = ohw.astype(ml_dtypes.bfloat16)
    return triu, ohw, ohwb, ltw, imask


def prepare_in_maps(queries, keys, values):
    q = np.ascontiguousarray(queries, dtype=np.float32).reshape(B, L, C)
    k = np.ascontiguousarray(keys, dtype=np.float32).reshape(B, L, C)
    v = np.ascontiguousarray(values, dtype=np.float32)            # [B,L,H,E]

    triu, ohw, ohwb, ltw, imask = _consts()
    # Zi[i] = q[i] for i<B else k[i-B]  -> [2B, L, C]
    Zi = np.concatenate([q, k], axis=0)

    in_maps = []
    for c in range(NCORES):
        b, half = c // 2, c % 2
        Zb = np.concatenate([q[b], k[b]], axis=0)                 # [4096, 512]
        own = Zb[2048 * half:2048 * half + 2048]
        oth = Zb[2048 * (1 - half):2048 * (1 - half) + 2048]
        # rotate other-half 512-blocks by `half` so the checkerboard rule
        # (r' + cpos even) covers complementary cross sub-blocks on the
        # two cores of a batch
        oth = np.concatenate(
            [oth[512 * ((i + half) % 4):512 * ((i + half) % 4) + 512]
             for i in range(4)], axis=0)
        zt = np.ascontiguousarray(
            np.concatenate([own, oth], axis=0).T).reshape(4, 128, T2)
        t0 = 256 * c
        zi = np.ascontiguousarray(
            Zi[:, t0:t0 + 256, :].transpose(1, 0, 2)).reshape(2, 128, 8, C)
        vp = np.ascontiguousarray(
            v[b].reshape(16, 128, H, E)[:, :, 4 * half:4 * half + 4, :]
            .transpose(1, 0, 2, 3))                               # [128,16,4,64]
        in_maps.append({
            "zt": zt, "zi": zi, "vp": vp,
            "triu": triu, "ohw": ohw, "ohwb": ohwb, "ltw": ltw,
            "imask": imask,
        })
    return in_maps


def get_nc():
    if "nc" not in _CACHED:
        _CACHED["nc"] = _build_nc()
    return _CACHED["nc"]


def kernel(queries, keys, values, attn_mask):
    global LAST_RUN
    nc = get_nc()
    in_maps = prepare_in_maps(queries, keys, values)

    res = run_bass_kernel_spmd(nc, in_maps, list(range(NCORES)))
    LAST_RUN = res
    results = res.results

    # ---- host combine (tiny) ----
    # temporal LSE: lse[r] = log(sum_n esums[p, m, n]) + SHIFT, r = 128m + p
    lse_t = np.zeros((B, 2, L))                                   # [b, half, t]
    dots = np.zeros((B, L))
    li_sum = np.zeros(L)                                          # sum_i LSE_inst
    pairs = [(i, j) for i in range(8) for j in range(i, 8)]
    pairs_rc = [(0, 1), (0, 2), (0, 3), (1, 2), (1, 3), (2, 3)]
    srows = np.zeros((B, 2, L))
    for c in range(NCORES):
        b, half = c // 2, c % 2
        r = results[c]
        es = r["esums"].astype(np.float64)                        # [128, 16, 3]
        mm, nnp = np.meshgrid(np.arange(16), np.arange(3), indexing="ij")
        keep = (nnp == 2) | (2 * nnp + 1 >= mm // 4)              # mask skipped
        s = (es * keep[None, :, :]).sum(axis=2)                   # [128, 16]
        srow = s.T.reshape(L).copy()                              # row r=128m+p
        cs = r["csums"].astype(np.float64)                        # [14, 512]
        # own-half mirrored lower super-blocks
        for p, (rr, ccol) in enumerate(pairs_rc):
            srow[512 * ccol:512 * ccol + 512] += cs[p]
        srows[b, half] += srow
        # cross checkerboard colsums belong to the *other* core's rows
        for rp in range(4):
            for hb in range(2):
                cpos = (rp % 2) + 2 * hb
                cact = (cpos + half) % 4
                srows[b, 1 - half, 512 * cact:512 * cact + 512] += cs[6 + 2 * rp + hb]
    for b in range(B):
        for half in range(2):
            lse_t[b, half] = np.log(srows[b, half]) + SHIFT

    for c in range(NCORES):
        b, half = c // 2, c % 2
        r = results[c]
        epk = r["eslab"].astype(np.float64).reshape(256, 36)      # [t, pair]
        e = np.empty((256, 8, 8))
        for p, (i, j) in enumerate(pairs):
            e[:, i, j] = epk[:, p]
            e[:, j, i] = epk[:, p]
        t0 = 256 * c
        for bb in range(B):
            dots[bb, t0:t0 + 256] = e[:, bb, 4 + bb]
        ii = np.arange(8)
        e[:, ii, ii] = -np.inf
        m = e.max(axis=2, keepdims=True)
        li = np.log(np.exp(e - m).sum(axis=2)) + m[..., 0]        # [256, 8]
        li_sum[t0:t0 + 256] = li.sum(axis=1)

    corr_mean = (li_sum + lse_t.sum(axis=(0, 1))) / 16.0 - dots.mean(axis=0)
    index = np.argsort(-corr_mean, kind="stable")[:TOPK]

    out = np.empty((B, H, L, E), np.float32)
    for c in range(NCORES):
        b, half = c // 2, c % 2
        pl = results[c]["planes"].reshape(128, 16, 4, E)
        out[b, 4 * half:4 * half + 4] = (
            pl.transpose(2, 1, 0, 3).reshape(4, L, E))
    out[:, :, index, :] /= (index + 1).astype(np.float32)[None, None, :, None]
    return out


# revision 38
# speedup vs baseline: 69.6757x; 1.0015x over previous
"""Trainium2 SPMD kernel for nn_AutoCorrelation_loss_V (sparse_attention).

Math summary (reference reduces to this exactly):
  - scores are constant along the unmasked (causal) key range, so softmax is
    uniform over l <= index[k]: attn @ V == prefix-mean of V at the selected rows.
  - the output is cumsum(V, axis=L) with the 7 selected rows divided by (idx+1).
  - the top-7 indices come from corr.mean(batch), where
      corr[b,t] = 0.25*(LSE_i1 + LSE_i2 + LSE_t1 + LSE_t2) - <q[b,t], k[b,t]>
    with LSE_t* = row-logsumexp (diag dropped) of the temporal Gram
    Z_b @ Z_b^T (Z_b = concat(q_b, k_b), [4096, 512]) and LSE_i* the row-LSE of
    the per-timestep 8x8 instance Gram.

Sharding (8 cores): core c = (b = c//2, half = c%2)
  - temporal Gram rows [2048*half : 2048*half+2048) of batch b: PE matmuls in
    float32r, fused exp(x - 100) + chunk-sum on ScalarE -> "esums" output.
  - instance grams for t in [256c, 256c+256): DVE rowwise-dot -> "eslab" output.
  - cumsum of V planes (b, heads 4*half..4*half+4) via triangular-ones matmuls
    (hierarchical: chunk-local cumsum + chunk-sum carry) -> "planes" output.
Host: combines the tiny LSE partials, takes top-7, divides those 7 rows by
(idx+1) while assembling the full [4, 8, 2048, 64] output.
"""

import sys

import numpy as np

sys.path.insert(0, "/opt/trn_rl_repo")

import concourse.bacc as bacc
import concourse.tile as tile
from concourse import mybir
from concourse.bass_utils import run_bass_kernel_spmd

F32 = mybir.dt.float32
F32R = mybir.dt.float32r
BF16 = mybir.dt.bfloat16

B, L, H, E = 4, 2048, 8, 64
C = H * E  # 512
T2 = 2 * L  # 4096
NCORES = 8
TOPK = 7  # int(1.0 * log(2048))
SHIFT = 100.0  # global exp shift; temporal Gram entries are in [-180, 180]

LAST_RUN = None  # BassKernelResults of the most recent launch (for test.py)

_CACHED = {}


def _build_nc():
    nc = bacc.Bacc("TRN2", target_bir_lowering=False, debug=False,
                   num_devices=NCORES)

    zt_d = nc.dram_tensor("zt", [4, 128, T2], F32R, kind="ExternalInput").ap()
    zi_d = nc.dram_tensor("zi", [2, 128, 8, C], F32, kind="ExternalInput").ap()
    vp_d = nc.dram_tensor("vp", [128, 16, 4, E], F32R, kind="ExternalInput").ap()
    triu_d = nc.dram_tensor("triu", [128, 128], F32R, kind="ExternalInput").ap()
    ohw_d = nc.dram_tensor("ohw", [128, 31], F32R, kind="ExternalInput").ap()
    ohwb_d = nc.dram_tensor("ohwb", [128, 31], BF16, kind="ExternalInput").ap()
    ltw_d = nc.dram_tensor("ltw", [16, 2048], F32R, kind="ExternalInput").ap()
    imask_d = nc.dram_tensor("imask", [128, 128], F32, kind="ExternalInput").ap()

    esums_d = nc.dram_tensor("esums", [128, 16, 3], F32, kind="ExternalOutput").ap()
    csums_d = nc.dram_tensor("csums", [14, 512], F32, kind="ExternalOutput").ap()
    eslab_d = nc.dram_tensor("eslab", [2, 128, 36], F32, kind="ExternalOutput").ap()
    planes_d = nc.dram_tensor("planes", [128, 16, 256], F32, kind="ExternalOutput").ap()

    with tile.TileContext(nc) as tc:
        with tc.tile_pool(name="zt", bufs=1) as ztp, \
             tc.tile_pool(name="zi", bufs=1) as zip_, \
             tc.tile_pool(name="vp", bufs=1) as vpp, \
             tc.tile_pool(name="const", bufs=1) as cp, \
             tc.tile_pool(name="small", bufs=1) as smp, \
             tc.tile_pool(name="scr", bufs=4) as scp, \
             tc.tile_pool(name="iscr", bufs=2) as iscp, \
             tc.tile_pool(name="osb", bufs=3) as osp, \
             tc.tile_pool(name="gram", bufs=2, space="PSUM") as gp, \
             tc.tile_pool(name="ghalf", bufs=3, space="PSUM") as gph, \
             tc.tile_pool(name="csp", bufs=1, space="PSUM") as csp:

            triu_sb = cp.tile([128, 128], F32R, tag="triu")
            nc.sync.dma_start(triu_sb[:], triu_d)
            ohw_sb = cp.tile([128, 31], F32R, tag="ohw")
            nc.sync.dma_start(ohw_sb[:], ohw_d)
            ohwb_sb = cp.tile([128, 31], BF16, tag="ohwb")
            nc.sync.dma_start(ohwb_sb[:], ohwb_d)
            ltw_sb = cp.tile([16, 2048], F32R, tag="ltw")
            nc.sync.dma_start(ltw_sb[:], ltw_d)
            imask_sb = cp.tile([128, 128], F32, tag="imask")
            nc.sync.dma_start(imask_sb[:], imask_d)
            bias_sb = cp.tile([128, 1], F32, tag="bias")
            nc.gpsimd.memset(bias_sb[:], -SHIFT)

            # DMA order tuned for overlap: zt chunk-columns n=0,1 first so the
            # temporal matmuls can start ~3us in; vp next (cumsum block runs
            # between temporal passes); the rest of zt streams behind.
            zt_sb = []
            for kk in range(4):
                t = ztp.tile([128, T2], F32R, tag=f"zt{kk}")
                zt_sb.append(t)

            def load_zt_chunk(n):
                for kk in range(4):
                    nc.sync.dma_start(zt_sb[kk][:, 512 * n:512 * n + 512],
                                      zt_d[kk, :, 512 * n:512 * n + 512])

            load_zt_chunk(0)
            load_zt_chunk(1)
            vp_sb = vpp.tile([128, 16, 4, E], F32R, tag="vp")
            nc.sync.dma_start(vp_sb[:], vp_d)
            load_zt_chunk(2)
            load_zt_chunk(3)
            zi_sb = []
            for tt in range(2):
                t = zip_.tile([128, 8, C], F32, tag=f"zi{tt}")
                nc.sync.dma_start(t[:], zi_d[tt])
                zi_sb.append(t)
            for n in range(4, 8):
                load_zt_chunk(n)

            esums_sb = smp.tile([128, 16, 3], F32, tag="esums")
            # colsum accumulator rows: 0..5 = own-half skipped lower
            # super-blocks (pair p = (r, c), r < c, row-major); 6..13 = cross
            # checkerboard sub-blocks (p = 6 + 2*r' + hb)
            pairs_rc = [(0, 1), (0, 2), (0, 3), (1, 2), (1, 3), (2, 3)]
            cs_ps = csp.tile([14, 512], F32, tag="csps")
            cs_state = {"first": True, "left": 24 + 32, "pending": []}

            def colsum_mm(p, rhs_ap):
                # deferred one unit so the PE (in-order) never waits on the
                # ACT exp that produces rhs
                cs_state["pending"].append((p, rhs_ap))

            def flush_colsums(keep=1):
                while len(cs_state["pending"]) > keep:
                    p, rhs_ap = cs_state["pending"].pop(0)
                    nc.tensor.matmul(cs_ps[:], ohwb_sb[:, 15 - p:29 - p],
                                     rhs_ap,
                                     start=cs_state["first"],
                                     stop=cs_state["left"] == 1,
                                     skip_group_check=True)
                    cs_state["first"] = False
                    cs_state["left"] -= 1

            # ---- temporal Gram + fused exp/chunk-sum, n-pair np at a time ----
            # Own-half columns (chunks 0..3) use upper-triangular symmetry:
            # chunk n < m//4 is skipped; its row-sums are recovered from
            # column-sums of the mirrored exp'd block (csums).
            def temporal_pass(np_, m_range=range(16)):
                for m in m_range:
                    g = m // 4
                    halves = [hn for hn in (0, 1) if 2 * np_ + hn >= g]
                    if not halves:
                        continue
                    flush_colsums(keep=4)
                    full = len(halves) == 2
                    width = 1024 if full else 512
                    ps = (gp.tile([128, width], F32, tag="gram", name="ps")
                          if full else
                          gph.tile([128, width], F32, tag="ghalf", name="ps"))
                    for hn in halves:
                        n = 2 * np_ + hn
                        col0 = 512 * (hn - halves[0])
                        for kk in range(4):
                            nc.tensor.matmul(
                                ps[:, col0:col0 + 512],
                                zt_sb[kk][:, 128 * m:128 * m + 128],
                                zt_sb[kk][:, 512 * n:512 * n + 512],
                                start=(kk == 0), stop=(kk == 3))
                    if 2 * np_ <= g <= 2 * np_ + 1:
                        # diagonal block lives in chunk n == g (first computed
                        # half): zero it so exp gives 0 (e^-100 underflows)
                        off = 128 * (m % 4)
                        nc.vector.tensor_mul(ps[:, off:off + 128],
                                             ps[:, off:off + 128], imask_sb[:])
                    scr = scp.tile([128, width], BF16,
                                   tag="scr" if full else "scrh")
                    nc.scalar.activation(scr[:], ps[:],
                                         mybir.ActivationFunctionType.Exp,
                                         bias=bias_sb[:],
                                         accum_out=esums_sb[:, m, np_:np_ + 1])
                    # column sums for strictly-upper own-half blocks (g < n < 4)
                    for hn in halves:
                        n = 2 * np_ + hn
                        if n < 4 and n > g:
                            p = pairs_rc.index((g, n))
                            col0 = 512 * (hn - halves[0])
                            colsum_mm(p, scr[:, col0:col0 + 512])

            # cross block (own rows x other-half cols): checkerboard — this
            # core computes col-positions cpos with (r' + cpos) even (the
            # other core's input rotation makes it cover the odd ones), and
            # emits colsums so the mirror rows are recovered host-side.
            def cross_pass():
                for m in range(16):
                    rp = m // 4
                    flush_colsums(keep=4)
                    ps = gp.tile([128, 1024], F32, tag="gram", name="ps")
                    for hb in range(2):
                        n = 4 + (rp % 2) + 2 * hb
                        for kk in range(4):
                            nc.tensor.matmul(
                                ps[:, 512 * hb:512 * hb + 512],
                                zt_sb[kk][:, 128 * m:128 * m + 128],
                                zt_sb[kk][:, 512 * n:512 * n + 512],
                                start=(kk == 0), stop=(kk == 3))
                    scr = scp.tile([128, 1024], BF16, tag="scr", name="scr")
                    nc.scalar.activation(scr[:], ps[:],
                                         mybir.ActivationFunctionType.Exp,
                                         bias=bias_sb[:],
                                         accum_out=esums_sb[:, m, 2:3])
                    for hb in range(2):
                        colsum_mm(6 + 2 * rp + hb,
                                  scr[:, 512 * hb:512 * hb + 512])

            temporal_pass(0)

            # ---- cumsum of V planes (PE-cheap; sits between temporal passes)
            ps_sums = gph.tile([16, 256], F32, tag="ghalf")
            for n in range(16):
                nc.tensor.matmul(ps_sums[:], ohw_sb[:, 15 - n:31 - n],
                                 vp_sb[:, n], start=(n == 0), stop=(n == 15))
            sums_sb = smp.tile([16, 256], F32R, tag="sums_sb")
            nc.scalar.copy(sums_sb[:], ps_sums[:])

            # a few pass-1 units run while the ACT chunk-sums copy lands, so
            # the carry matmuls below never stall the in-order PE
            temporal_pass(1, range(0, 4))

            for n in range(16):
                pc = gph.tile([128, 256], F32, tag="ghalf")
                nc.tensor.matmul(pc[:], ltw_sb[:, 128 * n:128 * n + 128],
                                 sums_sb[:], start=True, stop=False)
                nc.tensor.matmul(pc[:], triu_sb[:], vp_sb[:, n],
                                 start=False, stop=True)
                out_sb = osp.tile([128, 256], F32, tag="osb")
                nc.scalar.copy(out_sb[:], pc[:])
                nc.sync.dma_start(planes_d[:, n], out_sb[:])

            temporal_pass(1, range(4, 16))
            cross_pass()
            flush_colsums(keep=0)
            csums_sb = smp.tile([14, 512], F32, tag="csums_sb")
            nc.scalar.copy(csums_sb[:], cs_ps[:])
            nc.sync.dma_start(csums_d, csums_sb[:])
            nc.sync.dma_start(esums_d, esums_sb[:])

            # ---- instance grams: rowwise dots on DVE (36 pairs, host mirrors)
            pairs = [(i, j) for i in range(8) for j in range(i, 8)]
            for tt in range(2):
                eslab_sb = smp.tile([128, 36], F32, tag=f"eslab{tt}")
                for p, (i, j) in enumerate(pairs):
                    iscr = iscp.tile([128, C], F32, tag="iscr")
                    nc.vector.scalar_tensor_tensor(
                        iscr[:], zi_sb[tt][:, i, :], 1.0,
                        zi_sb[tt][:, j, :],
                        op0=mybir.AluOpType.mult,
                        op1=mybir.AluOpType.mult,
                        accum_out=eslab_sb[:, p:p + 1])
                nc.sync.dma_start(eslab_d[tt], eslab_sb[:])

    nc.compile()
    return nc


def _consts():
    k = np.arange(128)
    triu = (k[:, None] <= k[None, :]).astype(np.float32)          # lhsT cumsum
    ohw = np.zeros((128, 31), np.float32)
    ohw[:, 15] = 1.0                                              # one-hot cols
    cc = np.arange(16)
    nn = np.arange(2048) // 128
    ltw = (cc[:, None] < nn[None, :]).astype(np.float32)          # carry mask
    imask = (1.0 - np.eye(128)).astype(np.float32)
    import ml_dtypes
    ohwb = ohw.astype(ml_dtypes.bfloat16)
    return triu, ohw, ohwb, ltw, imask


def prepare_in_maps(queries, keys, values):
    q = np.ascontiguousarray(queries, dtype=np.float32).reshape(B, L, C)
    k = np.ascontiguousarray(keys, dtype=np.float32).reshape(B, L, C)
    v = np.ascontiguousarray(values, dtype=np.float32)            # [B,L,H,E]

    triu, ohw, ohwb, ltw, imask = _consts()
    # Zi[i] = q[i] for i<B else k[i-B]  -> [2B, L, C]
    Zi = np.concatenate([q, k], axis=0)

    in_maps = []
    for c in range(NCORES):
        b, half = c // 2, c % 2
        Zb = np.concatenate([q[b], k[b]], axis=0)                 # [4096, 512]
        own = Zb[2048 * half:2048 * half + 2048]
        oth = Zb[2048 * (1 - half):2048 * (1 - half) + 2048]
        # rotate other-half 512-blocks by `half` so the checkerboard rule
        # (r' + cpos even) covers complementary cross sub-blocks on the
        # two cores of a batch
        oth = np.concatenate(
            [oth[512 * ((i + half) % 4):512 * ((i + half) % 4) + 512]
             for i in range(4)], axis=0)
        zt = np.ascontiguousarray(
            np.concatenate([own, oth], axis=0).T).reshape(4, 128, T2)
        t0 = 256 * c
        zi = np.ascontiguousarray(
            Zi[:, t0:t0 + 256, :].transpose(1, 0, 2)).reshape(2, 128, 8, C)
        vp = np.ascontiguousarray(
            v[b].reshape(16, 128, H, E)[:, :, 4 * half:4 * half + 4, :]
            .transpose(1, 0, 2, 3))                               # [128,16,4,64]
        in_maps.append({
            "zt": zt, "zi": zi, "vp": vp,
            "triu": triu, "ohw": ohw, "ohwb": ohwb, "ltw": ltw,
            "imask": imask,
        })
    return in_maps


def get_nc():
    if "nc" not in _CACHED:
        _CACHED["nc"] = _build_nc()
    return _CACHED["nc"]


def kernel(queries, keys, values, attn_mask):
    global LAST_RUN
    nc = get_nc()
    in_maps = prepare_in_maps(queries, keys, values)

    res = run_bass_kernel_spmd(nc, in_maps, list(range(NCORES)))
    LAST_RUN = res
    results = res.results

    # ---- host combine (tiny) ----
    # temporal LSE: lse[r] = log(sum_n esums[p, m, n]) + SHIFT, r = 128m + p
    lse_t = np.zeros((B, 2, L))                                   # [b, half, t]
    dots = np.zeros((B, L))
    li_sum = np.zeros(L)                                          # sum_i LSE_inst
    pairs = [(i, j) for i in range(8) for j in range(i, 8)]
    pairs_rc = [(0, 1), (0, 2), (0, 3), (1, 2), (1, 3), (2, 3)]
    srows = np.zeros((B, 2, L))
    for c in range(NCORES):
        b, half = c // 2, c % 2
        r = results[c]
        es = r["esums"].astype(np.float64)                        # [128, 16, 3]
        mm, nnp = np.meshgrid(np.arange(16), np.arange(3), indexing="ij")
        keep = (nnp == 2) | (2 * nnp + 1 >= mm // 4)              # mask skipped
        s = (es * keep[None, :, :]).sum(axis=2)                   # [128, 16]
        srow = s.T.reshape(L).copy()                              # row r=128m+p
        cs = r["csums"].astype(np.float64)                        # [14, 512]
        # own-half mirrored lower super-blocks
        for p, (rr, ccol) in enumerate(pairs_rc):
            srow[512 * ccol:512 * ccol + 512] += cs[p]
        srows[b, half] += srow
        # cross checkerboard colsums belong to the *other* core's rows
        for rp in range(4):
            for hb in range(2):
                cpos = (rp % 2) + 2 * hb
                cact = (cpos + half) % 4
                srows[b, 1 - half, 512 * cact:512 * cact + 512] += cs[6 + 2 * rp + hb]
    for b in range(B):
        for half in range(2):
            lse_t[b, half] = np.log(srows[b, half]) + SHIFT

    for c in range(NCORES):
        b, half = c // 2, c % 2
        r = results[c]
        epk = r["eslab"].astype(np.float64).reshape(256, 36)      # [t, pair]
        e = np.empty((256, 8, 8))
        for p, (i, j) in enumerate(pairs):
            e[:, i, j] = epk[:, p]
            e[:, j, i] = epk[:, p]
        t0 = 256 * c
        for bb in range(B):
            dots[bb, t0:t0 + 256] = e[:, bb, 4 + bb]
        ii = np.arange(8)
        e[:, ii, ii] = -np.inf
        m = e.max(axis=2, keepdims=True)
        li = np.log(np.exp(e - m).sum(axis=2)) + m[..., 0]        # [256, 8]
        li_sum[t0:t0 + 256] = li.sum(axis=1)

    corr_mean = (li_sum + lse_t.sum(axis=(0, 1))) / 16.0 - dots.mean(axis=0)
    index = np.argsort(-corr_mean, kind="stable")[:TOPK]

    out = np.empty((B, H, L, E), np.float32)
    for c in range(NCORES):
        b, half = c // 2, c % 2
        pl = results[c]["planes"].reshape(128, 16, 4, E)
        out[b, 4 * half:4 * half + 4] = (
            pl.transpose(2, 1, 0, 3).reshape(4, L, E))
    out[:, :, index, :] /= (index + 1).astype(np.float32)[None, None, :, None]
    return out
